# revision 27
# baseline (speedup 1.0000x reference)
"""Trainium2 Bass kernel for nn_MultiHeadAttention_79465484911033.

Sharding: 8 cores = 2 batches x 4 head-groups (4 heads each of 16).
Each core: QKV projection for its heads (column-parallel), RoPE
(spatial+temporal angles composed into one rotation), causal attention,
swish, and a row-parallel partial output projection. Host sums the 4
partials per batch and adds b_out.

Device layout notes:
- x is passed pre-transposed (xt [H, L]) so the contraction dim is on
  partitions for every matmul.
- q/k are produced transposed ([chan, L]); v natural ([L, chan]).
- RoPE: q_rot = q*cos + (P@q)*sin where P is a pair-swap/sign matrix
  applied on the tensor engine; cos/sin are host-precomputed tables
  with spatial+temporal angles summed (rotations compose).
- Scores are computed transposed (s_T[k, q]), exp'd (no max subtraction
  -- scores are bounded) into bf16 prob tiles p_T[k, q].
- attn@v runs TRANSPOSED: stationary = v tile [128k, 65] (col 64 holds
  2.0), moving = p_T [128k, 512q] -> psum o_T [65, 512]; row 64
  accumulates 2*sum(p) = 2S. This streams 512 cols/matmul (vs 65 in a
  natural layout) and lands o directly in the layout oproj needs.
- swish(o/S) = t*(1+tanh(t)), t = o/(2S): reciprocal of row 64 (fast
  approx), partition-broadcast on the idle GPSIMD engine, one DVE mul,
  Tanh (shares the Exp activation-table set), one DVE STT into the
  transposed swished-output tile (aliased onto dead xt SBUF).
- oproj consumes the swished o_T blocks directly as stationary; no PE
  transposes needed anywhere in phase C.
"""

import sys

for _p in ("/opt/trn_rl_repo", "/root/.axon_site/_ro/trn_rl_repo"):
    if _p not in sys.path:
        sys.path.append(_p)

import numpy as np
import ml_dtypes

import concourse.bass as bass
import concourse.mybir as mybir
import concourse.tile as tile
from concourse import bacc
from concourse.bass_utils import run_bass_kernel_spmd

F32 = mybir.dt.float32
F32R = mybir.dt.float32r
BF16 = mybir.dt.bfloat16
AF = mybir.ActivationFunctionType
ALU = mybir.AluOpType

B, L, H = 2, 2048, 1024
NH, HD = 16, 64
NT, LS, L1D = 8, 256, 16
N_CORES = 8
HPC = 4               # heads per core
NKC = H // 128        # 8 contraction chunks
NL = L // 128         # 16 L chunks of 128
NLQ = L // 512        # 4 L tiles of 512

_CACHE = {}


def _build():
    nc = bacc.Bacc("TRN2", target_bir_lowering=False, debug=False,
                   enable_asserts=True, num_devices=N_CORES)

    xt_d = nc.dram_tensor("xt", [H, L], BF16, kind="ExternalInput")
    wqk_d = nc.dram_tensor("wqk", [H, 512], BF16, kind="ExternalInput")
    bqk_d = nc.dram_tensor("bqk", [128, 4], F32, kind="ExternalInput")
    wv_d = nc.dram_tensor("wv", [H, 256], BF16, kind="ExternalInput")
    bv_d = nc.dram_tensor("bv", [1, 256], BF16, kind="ExternalInput")
    cos_d = nc.dram_tensor("cosrep", [128, L], F32, kind="ExternalInput")
    sin_d = nc.dram_tensor("sinrep", [128, L], F32, kind="ExternalInput")
    pt_d = nc.dram_tensor("ptmat", [128, 128], F32R, kind="ExternalInput")
    tri_d = nc.dram_tensor("tri", [128, 128], BF16, kind="ExternalInput")
    wo_d = nc.dram_tensor("woT", [256, 1024], BF16, kind="ExternalInput")
    ones_d = nc.dram_tensor("ones1", [1, 128], BF16, kind="ExternalInput")
    out_d = nc.dram_tensor("out", [L, H], BF16, kind="ExternalOutput")

    with tile.TileContext(nc) as tc:
        with (
            tc.tile_pool(name="const", bufs=1) as cpool,
            tc.tile_pool(name="xt", bufs=1) as xpool,
            tc.tile_pool(name="w", bufs=1) as wpool,
            tc.tile_pool(name="qk", bufs=1) as qkpool,
            tc.tile_pool(name="v", bufs=1) as vpool,
            tc.tile_pool(name="work", bufs=1) as work,
            tc.tile_pool(name="pt", bufs=8) as ptpool,
            tc.tile_pool(name="sw", bufs=2) as swpool,
            tc.tile_pool(name="ps", bufs=1, space="PSUM") as psum,
        ):
            # ---- constants / weights to SBUF ----
            # DMA order = first-needed order; xt/cos/sin stream in 512-col
            # slabs so phase A/B start ~5us in.
            cos_t = cpool.tile([128, L], F32, tag="cos")
            sin_t = cpool.tile([128, L], F32, tag="sin")
            ptm_t = cpool.tile([128, 128], F32R, tag="ptm")
            tri_t = cpool.tile([128, 128], BF16, tag="tri")
            bqk_t = cpool.tile([128, 4], F32, tag="bqk")
            bv_t = cpool.tile([1, 256], BF16, tag="bv")
            ones_t = cpool.tile([1, 128], BF16, tag="ones")

            wv_t = []
            for k in range(NKC):
                t = wpool.tile([128, 256], BF16, tag=f"wv{k}", name=f"wv{k}")
                nc.sync.dma_start(t[:], wv_d[k * 128:(k + 1) * 128, :])
                wv_t.append(t)
            nc.sync.dma_start(bv_t[:], bv_d[:])
            nc.sync.dma_start(ones_t[:], ones_d[:])
            xt_t = [xpool.tile([128, L], BF16, tag=f"xt{k}", name=f"xt{k}")
                    for k in range(NKC)]
            wqk_t = [wpool.tile([128, 512], BF16, tag=f"wqk{k}",
                                name=f"wqk{k}") for k in range(NKC)]

            def dma_slab(s):
                sl = bass.ts(s, 512)
                for k in range(NKC):
                    nc.sync.dma_start(xt_t[k][:, sl],
                                      xt_d[k * 128:(k + 1) * 128, sl])
                nc.sync.dma_start(cos_t[:, sl], cos_d[:, sl])
                nc.sync.dma_start(sin_t[:, sl], sin_d[:, sl])

            dma_slab(0)
            for k in range(NKC):
                nc.sync.dma_start(wqk_t[k][:], wqk_d[k * 128:(k + 1) * 128, :])
            nc.sync.dma_start(bqk_t[:], bqk_d[:])
            nc.sync.dma_start(ptm_t[:], pt_d[:])
            for s in range(1, NLQ):
                dma_slab(s)
            nc.sync.dma_start(tri_t[:], tri_d[:])
            wo_t = []
            for g in range(2):
                t = wpool.tile([128, 1024], BF16, tag=f"wo{g}", name=f"wo{g}")
                nc.sync.dma_start(t[:], wo_d[g * 128:(g + 1) * 128, :])
                wo_t.append(t)

            # ---- PE warm-up: dummy matmuls while the first slab lands ----
            warm_dram = nc.dram_tensor("warm_scratch", [128, 128], F32,
                                       kind="Internal")
            wps = psum.tile([128, 512], F32, tag="op", bufs=2, name="warm_ps")
            NWARM = 36
            for w in range(NWARM):
                nc.tensor.matmul(wps[:, 0:256], wv_t[0][:, 0:128], wv_t[0][:],
                                 start=(w == 0), stop=(w == NWARM - 1))
            wsb = work.tile([128, 128], F32, tag="oT", bufs=2, name="warm_sb")
            nc.vector.tensor_copy(wsb[:], wps[:, 0:128])
            nc.sync.dma_start(warm_dram[:], wsb[:])

            # ---- phases A+B, slab-pipelined ----
            # A: v projection (natural) + bias, bf16, 2.0 col.
            # B: q/k projection (transposed) + bias + RoPE.
            # qkrot[m]: [128 chans, L]; m 0,1 = q (heads 0,1 | 2,3),
            # m 2,3 = k likewise.
            v_t = []
            for l in range(NL):
                t = vpool.tile([128, 4 * 65], BF16, tag=f"v{l}", name=f"v{l}")
                nc.vector.memset(
                    t[:].rearrange("p (h c) -> p h c", h=HPC)[:, :, 64:65], 2.0)
                v_t.append(t)
            qkrot = []
            for m in range(4):
                t = qkpool.tile([128, L], BF16, tag=f"qkr{m}", name=f"qkr{m}")
                qkrot.append(t)

            def phase_a_l(l):
                ps = psum.tile([128, 256], F32, tag="sc", bufs=2,
                               name=f"psv{l}")
                for k in range(NKC):
                    nc.tensor.matmul(
                        ps[:], xt_t[k][:, bass.ts(l, 128)], wv_t[k][:],
                        start=(k == 0), stop=False)
                nc.tensor.matmul(ps[:], ones_t[:], bv_t[:],
                                 start=False, stop=True)
                nc.vector.tensor_copy(
                    v_t[l][:].rearrange("p (h c) -> p h c", h=HPC)[:, :, 0:64],
                    ps[:].rearrange("p (h c) -> p h c", h=HPC))

            def phase_b_mn(m, n):
                sl = bass.ts(n, 512)
                qkb = work.tile([128, 512], F32R, tag="qkb", bufs=3,
                                name=f"qkb{m}_{n}")
                ps = psum.tile([128, 512], F32, tag="op", bufs=2,
                               name=f"psqk{m}_{n}")
                for k in range(NKC):
                    nc.tensor.matmul(
                        ps[:], wqk_t[k][:, bass.ts(m, 128)],
                        xt_t[k][:, sl],
                        start=(k == 0), stop=(k == NKC - 1))
                nc.scalar.add(qkb[:], ps[:], bqk_t[:, m:m + 1])
                # rope shuffle: sh = P @ qkb
                sh = psum.tile([128, 512], F32, tag="op", bufs=2,
                               name=f"pssh{m}_{n}")
                nc.tensor.matmul(sh[:], ptm_t[:], qkb[:],
                                 start=True, stop=True)
                # rot = qkb*cos + sh*sin (bf16 out)
                nc.vector.tensor_mul(qkrot[m][:, sl],
                                     qkb[:].bitcast(F32), cos_t[:, sl])
                nc.vector.tensor_mul(qkb[:], sh[:], sin_t[:, sl])
                nc.vector.tensor_add(qkrot[m][:, sl],
                                     qkrot[m][:, sl], qkb[:].bitcast(F32))



            # ---- phase C: attention, head-pair packed, transposed attn@v --
            # oT[g]: [128 chans, L] f32r swished attention output (transposed
            # layout), aliased onto the (dead after phase B) xt tiles.
            # Pair g = heads (2g, 2g+1): q = qkrot[g] (head 2g on partitions
            # 0-63, head 2g+1 on 64-127), k = qkrot[2+g] likewise. The two
            # score matmuls of a pair use disjoint PE row groups
            # (tile_position (0,0) / (64,0)) and run concurrently.
            # oT gets its own SBUF (bf16): xt tiles stay live for the
            # interleaved phase-B pair-1 chunks.
            oT = [qkpool.tile([128, L], BF16, tag=f"oT{g}", name=f"oT{g}")
                  for g in range(2)]
            LAG = 3

            def c_slot(g, i):
                # scores + exp + (lagged) attn@v for head pair g, q-tile i
                nj = 4 * i + 4
                q_t, k_t = qkrot[g], qkrot[2 + g]
                o_ps = psum.tile([65, 1024], F32, tag="oacc", bufs=1,
                                 name=f"oacc{g}_{i}")
                pts = []

                def av(j):
                    vf = max(0, j - 4 * i) * 128
                    for h in range(2):
                        nc.tensor.matmul(
                            o_ps[:, h * 512 + vf:h * 512 + 512],
                            v_t[j][:, (2 * g + h) * 65:(2 * g + h) * 65 + 65],
                            pts[j][:, h * 512 + vf:h * 512 + 512],
                            start=(j == 0), stop=(j == nj - 1),
                            skip_group_check=True)

                for j in range(nj):
                    d = j - 4 * i
                    ss = psum.tile([128, 1024], F32, tag="sc", bufs=2,
                                   name=f"ss{g}_{i}_{j}")
                    for h in range(2):
                        nc.tensor.matmul(
                            ss[:, h * 512:(h + 1) * 512],
                            k_t[h * 64:h * 64 + 64, bass.ts(j, 128)],
                            q_t[h * 64:h * 64 + 64, bass.ts(i, 512)],
                            start=True, stop=True)
                    pt = ptpool.tile([128, 1024], BF16, tag="pt",
                                     name=f"pt{g}_{i}_{j}")
                    vf = max(0, d) * 128
                    nc.scalar.activation(
                        pt[:].rearrange("p (h q) -> p h q", h=2)[:, :, vf:512],
                        ss[:].rearrange("p (h q) -> p h q", h=2)[:, :, vf:512],
                        AF.Exp, scale=0.125)
                    if d >= 0:
                        # causal mask inside the diagonal block: keep q >= k
                        # (iota = -p + c), on the otherwise-idle GPSIMD
                        nc.gpsimd.affine_select(
                            pt[:].rearrange(
                                "p (h q) -> p h q", h=2)[:, :, vf:vf + 128],
                            pt[:].rearrange(
                                "p (h q) -> p h q", h=2)[:, :, vf:vf + 128],
                            pattern=[[0, 2], [1, 128]],
                            compare_op=ALU.is_ge, fill=0.0,
                            base=0, channel_multiplier=-1)
                    pts.append(pt)
                    if j >= LAG:
                        av(j - LAG)
                for j in range(max(0, nj - LAG), nj):
                    av(j)

                # Drain psum to SBUF at once (frees the accumulator bank for
                # the next slot's attn@v), then swish off the critical path:
                # swish(o/S) = t*(1+tanh(t)), t = o/(2S); o_sb row 64 = 2S.
                # The reciprocal runs on [128, 8] (DMA-transposed) because
                # DVE reciprocal costs ~6.4ns per FREE element.
                o_sb = swpool.tile([65, 1024], F32, tag="osb",
                                   name=f"osb{g}_{i}")
                nc.vector.tensor_copy(o_sb[:], o_ps[:])
                s2t = swpool.tile([128, 8], F32, tag="s2t", name=f"s2t{g}_{i}")
                nc.sync.dma_start(s2t[:], o_sb[64:65, :])
                rect = swpool.tile([128, 8], F32, tag="rect",
                                   name=f"rect{g}_{i}")
                nc.vector.reciprocal(rect[:], s2t[:])
                rec = swpool.tile([1, 1024], F32, tag="rec",
                                  name=f"rec{g}_{i}")
                nc.sync.dma_start(rec[:], rect[:])
                bc = swpool.tile([64, 1024], F32, tag="bc", name=f"bc{g}_{i}")
                nc.gpsimd.partition_broadcast(bc[:], rec[:], channels=64)
                for h in range(2):
                    t = swpool.tile([64, 512], F32, tag=f"t{h}",
                                    name=f"t{g}_{i}_{h}")
                    nc.vector.tensor_mul(t[:], o_sb[0:64, bass.ts(h, 512)],
                                         bc[:, bass.ts(h, 512)])
                    th = swpool.tile([64, 512], F32, tag=f"th{h}",
                                     name=f"th{g}_{i}_{h}")
                    nc.scalar.activation(th[:], t[:], AF.Tanh)
                    nc.vector.scalar_tensor_tensor(
                        oT[g][h * 64:h * 64 + 64, bass.ts(i, 512)],
                        th[:], 1.0, t[:], op0=ALU.add, op1=ALU.mult)

            def oproj_block(i):
                # output projection; bf16 partial out (host sums in f32)
                for l in range(4 * i, 4 * i + 4):
                    ost = swpool.tile([128, 1024], BF16, tag="ost", bufs=3,
                                      name=f"ost{l}")
                    for n in range(2):
                        ps = psum.tile([128, 512], F32, tag="op", bufs=2,
                                       name=f"pso{l}_{n}")
                        for g in range(2):
                            nc.tensor.matmul(
                                ps[:], oT[g][:, bass.ts(l, 128)],
                                wo_t[g][:, bass.ts(n, 512)],
                                start=(g == 0), stop=(g == 1))
                        nc.vector.tensor_copy(ost[:, bass.ts(n, 512)], ps[:])
                    nc.sync.dma_start(out_d[bass.ts(l, 128), :], ost[:])

            # Slab-staggered: pair-0 attention for q-tile s starts right
            # after slab s's projections (it only needs k-slabs <= s), so
            # its exps overlap the next slab's projection matmuls.
            for s in range(NLQ):
                for l in range(4 * s, 4 * s + 4):
                    phase_a_l(l)
                phase_b_mn(0, s)
                phase_b_mn(2, s)
                if s == 2:
                    phase_b_mn(1, 0)
                    phase_b_mn(3, 0)
                c_slot(0, s)
            # C pair 1 with the remaining phase-B pair-1 chunks and oproj
            # as PE filler while ACT chews the exps.
            for i in range(NLQ):
                c_slot(1, i)
                if i < NLQ - 1:
                    phase_b_mn(1, i + 1)
                    phase_b_mn(3, i + 1)
                if i > 0:
                    oproj_block(i - 1)
            oproj_block(NLQ - 1)

    nc.compile()
    return nc


def _rope_tables():
    f2 = 1.0 / (10000.0 ** (np.arange(0, HD, 4, dtype=np.float64)[:HD // 4] / HD))
    s = np.arange(LS, dtype=np.float64)
    ang_s = np.zeros((LS, HD // 2), dtype=np.float64)
    ang_s[:, :HD // 4] = np.outer(s % L1D, f2)
    ang_s[:, HD // 4:] = np.outer(s // L1D, f2)
    f1 = 1.0 / (10000.0 ** (np.arange(0, HD, 2, dtype=np.float64) / HD))
    ang_t = np.outer(np.arange(NT, dtype=np.float64), f1)
    l = np.arange(L)
    ang = ang_s[l % LS] + ang_t[l // LS]        # [L, 32]
    pair = (np.arange(128) % HD) // 2           # [128] -> pair index
    cosrep = np.cos(ang).T[pair].astype(np.float32)  # [128, L]
    sinrep = np.sin(ang).T[pair].astype(np.float32)
    return np.ascontiguousarray(cosrep), np.ascontiguousarray(sinrep)


def _const_inputs():
    cosrep, sinrep = _rope_tables()
    ptmat = np.zeros((128, 128), dtype=np.float32)
    for i in range(64):
        ptmat[2 * i + 1, 2 * i] = -1.0   # shuffle[2i]   = -q[2i+1]
        ptmat[2 * i, 2 * i + 1] = 1.0    # shuffle[2i+1] = +q[2i]
    r = np.arange(128)
    tri = (r[None, :] >= r[:, None]).astype(ml_dtypes.bfloat16)  # [k, q]
    return cosrep, sinrep, ptmat, tri


def _make_in_maps(inp):
    x = np.asarray(inp["x"], dtype=np.float32)
    w_qkv = np.asarray(inp["w_qkv"], dtype=np.float32)
    b_qkv = np.asarray(inp["b_qkv"], dtype=np.float32)
    w_out = np.asarray(inp["w_out"], dtype=np.float32)
    if "consts" not in _CACHE:
        _CACHE["consts"] = _const_inputs()
    cosrep, sinrep, ptmat, tri = _CACHE["consts"]
    in_maps = []
    for c in range(N_CORES):
        b = c // 4
        heads = [4 * (c % 4) + i for i in range(HPC)]
        qrows = [h * 192 + j for h in heads for j in range(64)]
        krows = [h * 192 + 64 + j for h in heads for j in range(64)]
        vrows = [h * 192 + 128 + j for h in heads for j in range(64)]
        ocols = [h * 64 + j for h in heads for j in range(64)]
        bf = ml_dtypes.bfloat16
        wqk = np.ascontiguousarray(w_qkv[qrows + krows, :].T).astype(bf)
        bqk = np.ascontiguousarray(b_qkv[qrows + krows].reshape(4, 128).T)
        wv = np.ascontiguousarray(w_qkv[vrows, :].T).astype(bf)
        bv = np.ascontiguousarray(b_qkv[vrows].reshape(1, 256)).astype(bf)
        woT = np.ascontiguousarray(w_out[:, ocols].T).astype(bf)
        xt = np.ascontiguousarray(x[b].T).astype(bf)
        in_maps.append({
            "xt": xt, "wqk": wqk, "bqk": bqk, "wv": wv, "bv": bv,
            "cosrep": cosrep, "sinrep": sinrep, "ptmat": ptmat,
            "tri": tri, "woT": woT,
            "ones1": np.ones((1, 128), dtype=bf),
        })
    return in_maps


def kernel(x, w_qkv, b_qkv, w_out, b_out):
    b_out = np.asarray(b_out, dtype=np.float32)
    if "nc" not in _CACHE:
        _CACHE["nc"] = _build()
    nc = _CACHE["nc"]
    in_maps = _make_in_maps({"x": x, "w_qkv": w_qkv, "b_qkv": b_qkv,
                             "w_out": w_out})

    res = run_bass_kernel_spmd(nc, in_maps, core_ids=list(range(N_CORES)))

    out = np.zeros((B, L, H), dtype=np.float32)
    for c in range(N_CORES):
        out[c // 4] += res.results[c]["out"].astype(np.float32)
    out += b_out[None, None, :]
    return out


# revision 29
# speedup vs baseline: 1.0875x; 1.0875x over previous
"""Trainium2 Bass kernel for nn_MultiHeadAttention_79465484911033.

Sharding: 8 cores = 2 batches x 4 head-groups (4 heads each of 16).
Each core: QKV projection for its heads (column-parallel), RoPE
(spatial+temporal angles composed into one rotation), causal attention,
swish, and a row-parallel partial output projection. Host sums the 4
partials per batch and adds b_out.

Device layout notes:
- x is passed pre-transposed (xt [H, L]) so the contraction dim is on
  partitions for every matmul.
- q/k are produced transposed ([chan, L]); v natural ([L, chan]).
- RoPE: q_rot = q*cos + (P@q)*sin where P is a pair-swap/sign matrix
  applied on the tensor engine; cos/sin are host-precomputed tables
  with spatial+temporal angles summed (rotations compose).
- Scores are computed transposed (s_T[k, q]), exp'd (no max subtraction
  -- scores are bounded) into bf16 prob tiles p_T[k, q].
- attn@v runs TRANSPOSED: stationary = v tile [128k, 65] (col 64 holds
  2.0), moving = p_T [128k, 512q] -> psum o_T [65, 512]; row 64
  accumulates 2*sum(p) = 2S. This streams 512 cols/matmul (vs 65 in a
  natural layout) and lands o directly in the layout oproj needs.
- swish(o/S) = t*(1+tanh(t)), t = o/(2S): reciprocal of row 64 (fast
  approx), partition-broadcast on the idle GPSIMD engine, one DVE mul,
  Tanh (shares the Exp activation-table set), one DVE STT into the
  transposed swished-output tile (aliased onto dead xt SBUF).
- oproj consumes the swished o_T blocks directly as stationary; no PE
  transposes needed anywhere in phase C.
"""

import sys

for _p in ("/opt/trn_rl_repo", "/root/.axon_site/_ro/trn_rl_repo"):
    if _p not in sys.path:
        sys.path.append(_p)

import numpy as np
import ml_dtypes

import concourse.bass as bass
import concourse.mybir as mybir
import concourse.tile as tile
from concourse import bacc
from concourse.bass_utils import run_bass_kernel_spmd

F32 = mybir.dt.float32
F32R = mybir.dt.float32r
BF16 = mybir.dt.bfloat16
AF = mybir.ActivationFunctionType
ALU = mybir.AluOpType

B, L, H = 2, 2048, 1024
NH, HD = 16, 64
NT, LS, L1D = 8, 256, 16
N_CORES = 8
HPC = 4               # heads per core
NKC = H // 128        # 8 contraction chunks
NL = L // 128         # 16 L chunks of 128
NLQ = L // 512        # 4 L tiles of 512

_CACHE = {}


def _build():
    nc = bacc.Bacc("TRN2", target_bir_lowering=False, debug=False,
                   enable_asserts=True, num_devices=N_CORES)

    xt_d = nc.dram_tensor("xt", [H, L], BF16, kind="ExternalInput")
    wqk_d = nc.dram_tensor("wqk", [H, 512], BF16, kind="ExternalInput")
    bqk_d = nc.dram_tensor("bqk", [128, 4], F32, kind="ExternalInput")
    wv_d = nc.dram_tensor("wv", [H, 256], BF16, kind="ExternalInput")
    bv_d = nc.dram_tensor("bv", [1, 256], BF16, kind="ExternalInput")
    cos_d = nc.dram_tensor("cosrep", [128, L], F32, kind="ExternalInput")
    sin_d = nc.dram_tensor("sinrep", [128, L], F32, kind="ExternalInput")
    pt_d = nc.dram_tensor("ptmat", [128, 128], F32R, kind="ExternalInput")
    tri_d = nc.dram_tensor("tri", [128, 128], BF16, kind="ExternalInput")
    wo_d = nc.dram_tensor("woT", [256, 1024], BF16, kind="ExternalInput")
    ones_d = nc.dram_tensor("ones1", [1, 128], BF16, kind="ExternalInput")
    out_d = nc.dram_tensor("out", [L, H], BF16, kind="ExternalOutput")

    with tile.TileContext(nc) as tc:
        with (
            tc.tile_pool(name="const", bufs=1) as cpool,
            tc.tile_pool(name="xt", bufs=1) as xpool,
            tc.tile_pool(name="w", bufs=1) as wpool,
            tc.tile_pool(name="qk", bufs=1) as qkpool,
            tc.tile_pool(name="v", bufs=1) as vpool,
            tc.tile_pool(name="work", bufs=1) as work,
            tc.tile_pool(name="pt", bufs=8) as ptpool,
            tc.tile_pool(name="sw", bufs=2) as swpool,
            tc.tile_pool(name="ps", bufs=1, space="PSUM") as psum,
        ):
            # ---- constants / weights to SBUF ----
            # DMA order = first-needed order; xt/cos/sin stream in 512-col
            # slabs so phase A/B start ~5us in.
            cos_t = cpool.tile([128, L], F32, tag="cos")
            sin_t = cpool.tile([128, L], F32, tag="sin")
            ptm_t = cpool.tile([128, 128], F32R, tag="ptm")
            tri_t = cpool.tile([128, 128], BF16, tag="tri")
            bqk_t = cpool.tile([128, 4], F32, tag="bqk")
            bv_t = cpool.tile([1, 256], BF16, tag="bv")
            ones_t = cpool.tile([1, 128], BF16, tag="ones")

            wv_t = []
            for k in range(NKC):
                t = wpool.tile([128, 256], BF16, tag=f"wv{k}", name=f"wv{k}")
                nc.sync.dma_start(t[:], wv_d[k * 128:(k + 1) * 128, :])
                wv_t.append(t)
            nc.sync.dma_start(bv_t[:], bv_d[:])
            nc.sync.dma_start(ones_t[:], ones_d[:])
            xt_t = [xpool.tile([128, L], BF16, tag=f"xt{k}", name=f"xt{k}")
                    for k in range(NKC)]
            wqk_t = [wpool.tile([128, 512], BF16, tag=f"wqk{k}",
                                name=f"wqk{k}") for k in range(NKC)]

            def dma_slab(s):
                sl = bass.ts(s, 512)
                for k in range(NKC):
                    nc.sync.dma_start(xt_t[k][:, sl],
                                      xt_d[k * 128:(k + 1) * 128, sl])
                nc.sync.dma_start(cos_t[:, sl], cos_d[:, sl])
                nc.sync.dma_start(sin_t[:, sl], sin_d[:, sl])

            dma_slab(0)
            for k in range(NKC):
                nc.sync.dma_start(wqk_t[k][:], wqk_d[k * 128:(k + 1) * 128, :])
            nc.sync.dma_start(bqk_t[:], bqk_d[:])
            nc.sync.dma_start(ptm_t[:], pt_d[:])
            for s in range(1, NLQ):
                dma_slab(s)
            nc.sync.dma_start(tri_t[:], tri_d[:])
            wo_t = []
            for g in range(2):
                t = wpool.tile([128, 1024], BF16, tag=f"wo{g}", name=f"wo{g}")
                nc.sync.dma_start(t[:], wo_d[g * 128:(g + 1) * 128, :])
                wo_t.append(t)

            # ---- PE warm-up: dummy matmuls while the first slab lands ----
            warm_dram = nc.dram_tensor("warm_scratch", [128, 128], F32,
                                       kind="Internal")
            wps = psum.tile([128, 512], F32, tag="op", bufs=2, name="warm_ps")
            NWARM = 36
            for w in range(NWARM):
                nc.tensor.matmul(wps[:, 0:256], wv_t[0][:, 0:128], wv_t[0][:],
                                 start=(w == 0), stop=(w == NWARM - 1))
            wsb = work.tile([128, 128], F32, tag="oT", bufs=2, name="warm_sb")
            nc.vector.tensor_copy(wsb[:], wps[:, 0:128])
            nc.sync.dma_start(warm_dram[:], wsb[:])

            # ---- phases A+B, slab-pipelined ----
            # A: v projection (natural) + bias, bf16, 2.0 col.
            # B: q/k projection (transposed) + bias + RoPE.
            # qkrot[m]: [128 chans, L]; m 0,1 = q (heads 0,1 | 2,3),
            # m 2,3 = k likewise.
            v_t = []
            for l in range(NL):
                t = vpool.tile([128, 4 * 65], BF16, tag=f"v{l}", name=f"v{l}")
                nc.vector.memset(
                    t[:].rearrange("p (h c) -> p h c", h=HPC)[:, :, 64:65], 2.0)
                v_t.append(t)
            qkrot = []
            for m in range(4):
                t = qkpool.tile([128, L], BF16, tag=f"qkr{m}", name=f"qkr{m}")
                qkrot.append(t)

            def phase_a_l(l):
                ps = psum.tile([128, 256], F32, tag="sc", bufs=2,
                               name=f"psv{l}")
                for k in range(NKC):
                    nc.tensor.matmul(
                        ps[:], xt_t[k][:, bass.ts(l, 128)], wv_t[k][:],
                        start=(k == 0), stop=False)
                nc.tensor.matmul(ps[:], ones_t[:], bv_t[:],
                                 start=False, stop=True)
                nc.vector.tensor_copy(
                    v_t[l][:].rearrange("p (h c) -> p h c", h=HPC)[:, :, 0:64],
                    ps[:].rearrange("p (h c) -> p h c", h=HPC))

            def phase_b_mn(m, n):
                sl = bass.ts(n, 512)
                qkb = work.tile([128, 512], F32R, tag="qkb", bufs=3,
                                name=f"qkb{m}_{n}")
                ps = psum.tile([128, 512], F32, tag="op", bufs=2,
                               name=f"psqk{m}_{n}")
                for k in range(NKC):
                    nc.tensor.matmul(
                        ps[:], wqk_t[k][:, bass.ts(m, 128)],
                        xt_t[k][:, sl],
                        start=(k == 0), stop=(k == NKC - 1))
                nc.scalar.add(qkb[:], ps[:], bqk_t[:, m:m + 1])
                # rope shuffle: sh = P @ qkb
                sh = psum.tile([128, 512], F32, tag="op", bufs=2,
                               name=f"pssh{m}_{n}")
                nc.tensor.matmul(sh[:], ptm_t[:], qkb[:],
                                 start=True, stop=True)
                # rot = qkb*cos + sh*sin (bf16 out)
                nc.vector.tensor_mul(qkrot[m][:, sl],
                                     qkb[:].bitcast(F32), cos_t[:, sl])
                nc.vector.tensor_mul(qkb[:], sh[:], sin_t[:, sl])
                nc.vector.tensor_add(qkrot[m][:, sl],
                                     qkrot[m][:, sl], qkb[:].bitcast(F32))



            # ---- phase C: attention, head-pair packed, transposed attn@v --
            # oT[g]: [128 chans, L] f32r swished attention output (transposed
            # layout), aliased onto the (dead after phase B) xt tiles.
            # Pair g = heads (2g, 2g+1): q = qkrot[g] (head 2g on partitions
            # 0-63, head 2g+1 on 64-127), k = qkrot[2+g] likewise. The two
            # score matmuls of a pair use disjoint PE row groups
            # (tile_position (0,0) / (64,0)) and run concurrently.
            # oT gets its own SBUF (bf16): xt tiles stay live for the
            # interleaved phase-B pair-1 chunks.
            oT = [qkpool.tile([128, L], BF16, tag=f"oT{g}", name=f"oT{g}")
                  for g in range(2)]
            LAG = 3

            def c_slot(g, i):
                # scores + exp + (lagged) attn@v for head pair g, q-tile i
                nj = 4 * i + 4
                q_t, k_t = qkrot[g], qkrot[2 + g]
                o_ps = psum.tile([65, 1024], F32, tag="oacc", bufs=1,
                                 name=f"oacc{g}_{i}")
                pts = []

                def av(j):
                    vf = max(0, j - 4 * i) * 128
                    for h in range(2):
                        nc.tensor.matmul(
                            o_ps[:, h * 512 + vf:h * 512 + 512],
                            v_t[j][:, (2 * g + h) * 65:(2 * g + h) * 65 + 65],
                            pts[j][:, h * 512 + vf:h * 512 + 512],
                            start=(j == 0), stop=(j == nj - 1),
                            skip_group_check=True)

                for j in range(nj):
                    d = j - 4 * i
                    ss = psum.tile([128, 1024], F32, tag="sc", bufs=2,
                                   name=f"ss{g}_{i}_{j}")
                    for h in range(2):
                        nc.tensor.matmul(
                            ss[:, h * 512:(h + 1) * 512],
                            k_t[h * 64:h * 64 + 64, bass.ts(j, 128)],
                            q_t[h * 64:h * 64 + 64, bass.ts(i, 512)],
                            start=True, stop=True)
                    pt = ptpool.tile([128, 1024], BF16, tag="pt",
                                     name=f"pt{g}_{i}_{j}")
                    vf = max(0, d) * 128
                    nc.scalar.activation(
                        pt[:].rearrange("p (h q) -> p h q", h=2)[:, :, vf:512],
                        ss[:].rearrange("p (h q) -> p h q", h=2)[:, :, vf:512],
                        AF.Exp, scale=0.125)
                    if d >= 0:
                        # causal mask inside the diagonal block: keep q >= k
                        # (iota = -p + c), on the otherwise-idle GPSIMD
                        nc.gpsimd.affine_select(
                            pt[:].rearrange(
                                "p (h q) -> p h q", h=2)[:, :, vf:vf + 128],
                            pt[:].rearrange(
                                "p (h q) -> p h q", h=2)[:, :, vf:vf + 128],
                            pattern=[[0, 2], [1, 128]],
                            compare_op=ALU.is_ge, fill=0.0,
                            base=0, channel_multiplier=-1)
                    pts.append(pt)
                    if j >= LAG:
                        av(j - LAG)
                for j in range(max(0, nj - LAG), nj):
                    av(j)

                # Drain psum to SBUF at once (frees the accumulator bank for
                # the next slot's attn@v), then swish off the critical path:
                # swish(o/S) = t*(1+tanh(t)), t = o/(2S); o_sb row 64 = 2S.
                # The reciprocal runs on [128, 8] (DMA-transposed) because
                # DVE reciprocal costs ~6.4ns per FREE element.
                o_sb = swpool.tile([65, 1024], F32, tag="osb",
                                   name=f"osb{g}_{i}")
                nc.vector.tensor_copy(o_sb[:], o_ps[:])
                s2t = swpool.tile([128, 8], F32, tag="s2t", name=f"s2t{g}_{i}")
                nc.sync.dma_start(s2t[:], o_sb[64:65, :])
                rect = swpool.tile([128, 8], F32, tag="rect",
                                   name=f"rect{g}_{i}")
                nc.vector.reciprocal(rect[:], s2t[:])
                rec = swpool.tile([1, 1024], F32, tag="rec",
                                  name=f"rec{g}_{i}")
                nc.sync.dma_start(rec[:], rect[:])
                bc = swpool.tile([64, 1024], F32, tag="bc", name=f"bc{g}_{i}")
                nc.gpsimd.partition_broadcast(bc[:], rec[:], channels=64)
                for h in range(2):
                    t = swpool.tile([64, 512], F32, tag=f"t{h}",
                                    name=f"t{g}_{i}_{h}")
                    nc.vector.tensor_mul(t[:], o_sb[0:64, bass.ts(h, 512)],
                                         bc[:, bass.ts(h, 512)])
                    th = swpool.tile([64, 512], F32, tag=f"th{h}",
                                     name=f"th{g}_{i}_{h}")
                    nc.scalar.activation(th[:], t[:], AF.Tanh)
                    nc.vector.scalar_tensor_tensor(
                        oT[g][h * 64:h * 64 + 64, bass.ts(i, 512)],
                        th[:], 1.0, t[:], op0=ALU.add, op1=ALU.mult)

            def oproj_block(i):
                # output projection; bf16 partial out (host sums in f32)
                for l in range(4 * i, 4 * i + 4):
                    ost = swpool.tile([128, 1024], BF16, tag="ost", bufs=3,
                                      name=f"ost{l}")
                    for n in range(2):
                        ps = psum.tile([128, 512], F32, tag="op", bufs=2,
                                       name=f"pso{l}_{n}")
                        for g in range(2):
                            nc.tensor.matmul(
                                ps[:], oT[g][:, bass.ts(l, 128)],
                                wo_t[g][:, bass.ts(n, 512)],
                                start=(g == 0), stop=(g == 1))
                        nc.vector.tensor_copy(ost[:, bass.ts(n, 512)], ps[:])
                    nc.sync.dma_start(out_d[bass.ts(l, 128), :], ost[:])

            # Phases A + B-pair0 slab-major, then C pair 0 with phase-B
            # pair-1 chunks interleaved to keep the PE fed while ACT chews
            # the exps, then C pair 1 with oproj lagged one tile behind.
            for s in range(NLQ):
                for l in range(4 * s, 4 * s + 4):
                    phase_a_l(l)
                phase_b_mn(0, s)
                phase_b_mn(2, s)
                # pair-0 attention for old q-tiles (deps are slabs-old, so
                # no rope stall): their exps overlap the next slab's PE work
                if s == 2:
                    c_slot(0, 0)
                if s == 3:
                    c_slot(0, 1)
            for i in range(2, NLQ):
                c_slot(0, i)
                phase_b_mn(1, 2 * (i - 2))
                phase_b_mn(3, 2 * (i - 2))
                phase_b_mn(1, 2 * (i - 2) + 1)
                phase_b_mn(3, 2 * (i - 2) + 1)
            for i in range(NLQ):
                c_slot(1, i)
                if i > 0:
                    oproj_block(i - 1)
            oproj_block(NLQ - 1)

    nc.compile()
    return nc


def _rope_tables():
    f2 = 1.0 / (10000.0 ** (np.arange(0, HD, 4, dtype=np.float64)[:HD // 4] / HD))
    s = np.arange(LS, dtype=np.float64)
    ang_s = np.zeros((LS, HD // 2), dtype=np.float64)
    ang_s[:, :HD // 4] = np.outer(s % L1D, f2)
    ang_s[:, HD // 4:] = np.outer(s // L1D, f2)
    f1 = 1.0 / (10000.0 ** (np.arange(0, HD, 2, dtype=np.float64) / HD))
    ang_t = np.outer(np.arange(NT, dtype=np.float64), f1)
    l = np.arange(L)
    ang = ang_s[l % LS] + ang_t[l // LS]        # [L, 32]
    pair = (np.arange(128) % HD) // 2           # [128] -> pair index
    cosrep = np.cos(ang).T[pair].astype(np.float32)  # [128, L]
    sinrep = np.sin(ang).T[pair].astype(np.float32)
    return np.ascontiguousarray(cosrep), np.ascontiguousarray(sinrep)


def _const_inputs():
    cosrep, sinrep = _rope_tables()
    ptmat = np.zeros((128, 128), dtype=np.float32)
    for i in range(64):
        ptmat[2 * i + 1, 2 * i] = -1.0   # shuffle[2i]   = -q[2i+1]
        ptmat[2 * i, 2 * i + 1] = 1.0    # shuffle[2i+1] = +q[2i]
    r = np.arange(128)
    tri = (r[None, :] >= r[:, None]).astype(ml_dtypes.bfloat16)  # [k, q]
    return cosrep, sinrep, ptmat, tri


def _make_in_maps(inp):
    x = np.asarray(inp["x"], dtype=np.float32)
    w_qkv = np.asarray(inp["w_qkv"], dtype=np.float32)
    b_qkv = np.asarray(inp["b_qkv"], dtype=np.float32)
    w_out = np.asarray(inp["w_out"], dtype=np.float32)
    if "consts" not in _CACHE:
        _CACHE["consts"] = _const_inputs()
    cosrep, sinrep, ptmat, tri = _CACHE["consts"]
    in_maps = []
    for c in range(N_CORES):
        b = c // 4
        heads = [4 * (c % 4) + i for i in range(HPC)]
        qrows = [h * 192 + j for h in heads for j in range(64)]
        krows = [h * 192 + 64 + j for h in heads for j in range(64)]
        vrows = [h * 192 + 128 + j for h in heads for j in range(64)]
        ocols = [h * 64 + j for h in heads for j in range(64)]
        bf = ml_dtypes.bfloat16
        wqk = np.ascontiguousarray(w_qkv[qrows + krows, :].T).astype(bf)
        bqk = np.ascontiguousarray(b_qkv[qrows + krows].reshape(4, 128).T)
        wv = np.ascontiguousarray(w_qkv[vrows, :].T).astype(bf)
        bv = np.ascontiguousarray(b_qkv[vrows].reshape(1, 256)).astype(bf)
        woT = np.ascontiguousarray(w_out[:, ocols].T).astype(bf)
        xt = np.ascontiguousarray(x[b].T).astype(bf)
        in_maps.append({
            "xt": xt, "wqk": wqk, "bqk": bqk, "wv": wv, "bv": bv,
            "cosrep": cosrep, "sinrep": sinrep, "ptmat": ptmat,
            "tri": tri, "woT": woT,
            "ones1": np.ones((1, 128), dtype=bf),
        })
    return in_maps


def kernel(x, w_qkv, b_qkv, w_out, b_out):
    b_out = np.asarray(b_out, dtype=np.float32)
    if "nc" not in _CACHE:
        _CACHE["nc"] = _build()
    nc = _CACHE["nc"]
    in_maps = _make_in_maps({"x": x, "w_qkv": w_qkv, "b_qkv": b_qkv,
                             "w_out": w_out})

    res = run_bass_kernel_spmd(nc, in_maps, core_ids=list(range(N_CORES)))

    out = np.zeros((B, L, H), dtype=np.float32)
    for c in range(N_CORES):
        out[c // 4] += res.results[c]["out"].astype(np.float32)
    out += b_out[None, None, :]
    return out
